# revision 1
# baseline (speedup 1.0000x reference)
"""Trainium2 Bass kernel for nn_CriticGCN (2-layer GCN critic, 50000 nodes,
800000 edges, 8 NeuronCores).

Algebraic reformulation (exact):
  A = S_dT diag(dis_s*dis_d) S_s + diag(dis^2)   (GCN norm adjacency)
  layer1: out1 = A @ (X W1) + b1 = ((A @ X) W1) + b1        (associativity)
  x1 = relu(out1); y = (A @ (x1 W2) + b2) @ W3 + b3
     = A @ (x1 (W2 W3)) + (b2 W3 + b3)                       (linearity)
  and A @ v = dis * (segsum_dst(dis_src * v[src]) + dis * v) per column.

So the device does two sparse passes (20-dim then 1-dim features) plus a
small dense chain; W2@W3 collapses layer 2's feature dim to 1.

Sharding: nodes are relabeled by a balance permutation and dst-sharded
across 8 cores (49 windows of 128 labels each). Edges are grouped per
(window, src<32768) and padded to fixed tile counts; per-edge rows are
fetched with dma_gather (256B rows from an HBM table), segment-summed via
one-hot matmuls on the TensorEngine accumulating in PSUM.
"""
import numpy as np
import concourse.bacc as bacc
import concourse.mybir as mybir
import concourse.tile as tile
from concourse.bass_utils import run_bass_kernel_spmd

P = 128
NCORES = 8
WPC = 49
NWIN = NCORES * WPC
NPAD = NWIN * P
GAP = 32767
NROWS = NPAD + 2
D = 64
DF = 20
GRP = 4
NGRP = (WPC + GRP - 1) // GRP
ZCOLS = NGRP * GRP * P

IOTA = np.broadcast_to(np.arange(P, dtype=np.float32)[None, :], (P, P)).copy()
IDENT = np.eye(P, dtype=np.float32)


def _row_of(lbl):
    return np.where(lbl < GAP, lbl, lbl + 1)


def _preprocess(state, edge_attr, edge_index):
    X = np.concatenate([state.reshape(-1, edge_attr.shape[1]),
                        edge_attr], 0).astype(np.float32)
    n = X.shape[0]
    src = edge_index[0].astype(np.int64)
    dst = edge_index[1].astype(np.int64)

    deg = np.bincount(dst, minlength=n) + 1
    dis = (1.0 / np.sqrt(deg)).astype(np.float32)

    order = np.argsort(-deg, kind="stable")
    sigma = np.empty(n, dtype=np.int64)
    sigma[order] = (np.arange(n) % NWIN) * P + np.arange(n) // NWIN

    s_row = _row_of(sigma[src])
    d_lbl = sigma[dst]
    d_win = d_lbl // P
    d_loc = d_lbl % P
    is_hi = s_row >= GAP + 1

    key = d_win * 2 + is_hi
    eorder = np.argsort(key, kind="stable")
    s_row_s = s_row[eorder]
    d_loc_s = d_loc[eorder]
    counts = np.bincount(key[eorder], minlength=NWIN * 2)
    off = np.concatenate([[0], np.cumsum(counts)])
    t_lo = int(np.ceil(counts[0::2].max() / P))
    t_hi = int(np.ceil(counts[1::2].max() / P))
    nlo, nhi = t_lo * P, t_hi * P

    idx_lo = np.full((NWIN, nlo), GAP, dtype=np.int64)
    idx_hi = np.full((NWIN, nhi), NPAD + 1 - (GAP + 1), dtype=np.int64)
    dloc = np.zeros((NWIN, (t_lo + t_hi) * P), dtype=np.int64)
    for w in range(NWIN):
        lo0, lo1 = off[2 * w], off[2 * w + 1]
        hi0, hi1 = off[2 * w + 1], off[2 * w + 2]
        klo, khi = lo1 - lo0, hi1 - hi0
        idx_lo[w, :klo] = s_row_s[lo0:lo1]
        idx_hi[w, :khi] = s_row_s[hi0:hi1] - (GAP + 1)
        dloc[w, :klo] = d_loc_s[lo0:lo1]
        dloc[w, nlo:nlo + khi] = d_loc_s[hi0:hi1]

    Xpad = np.zeros((NROWS, D), dtype=np.float32)
    rows = _row_of(sigma)
    Xpad[rows, :DF] = X
    Xpad[rows, DF] = dis

    dis_lbl = np.zeros(NPAD, dtype=np.float32)
    dis_lbl[sigma] = dis
    U_lbl = np.zeros((NPAD, DF), dtype=np.float32)
    U_lbl[sigma] = X * dis[:, None]

    def wrap16(a):
        return np.tile(a.astype(np.int16).reshape(-1, 16).T, (8, 1))

    per_core = []
    for k in range(NCORES):
        wr = range(WPC * k, WPC * (k + 1))
        ilo = np.concatenate([wrap16(idx_lo[w]) for w in wr], axis=1)
        ihi = np.concatenate([wrap16(idx_hi[w]) for w in wr], axis=1)
        dl = np.concatenate(
            [dloc[w].reshape(t_lo + t_hi, P).T.astype(np.float32) for w in wr],
            axis=1)
        lblk = np.arange(WPC * P) + WPC * k * P
        disw = dis_lbl[lblk].reshape(WPC, P).T.copy()
        Uw = U_lbl[lblk].reshape(WPC, P, DF).transpose(1, 0, 2).copy()
        per_core.append(dict(ilo=ilo, ihi=ihi, dloc=dl, disw=disw, U=Uw))
    return dict(per_core=per_core, Xpad=Xpad, sigma=sigma, dis_lbl=dis_lbl,
                t_lo=t_lo, t_hi=t_hi)


def build_pass1(t_lo, t_hi, reps=1):
    nlo, nhi = t_lo * P, t_hi * P
    t_w = t_lo + t_hi
    nc = bacc.Bacc("TRN2", target_bir_lowering=False, debug=False,
                   num_devices=NCORES)
    f32 = mybir.dt.float32
    xpad_d = nc.dram_tensor("xpad", [NROWS, D], f32, kind="ExternalInput")
    ilo_d = nc.dram_tensor("ilo", [128, WPC * nlo // 16], mybir.dt.int16, kind="ExternalInput")
    ihi_d = nc.dram_tensor("ihi", [128, WPC * nhi // 16], mybir.dt.int16, kind="ExternalInput")
    dloc_d = nc.dram_tensor("dloc", [P, WPC * t_w], f32, kind="ExternalInput")
    iota_d = nc.dram_tensor("iota", [P, P], f32, kind="ExternalInput")
    ident_d = nc.dram_tensor("ident", [P, P], f32, kind="ExternalInput")
    disw_d = nc.dram_tensor("disw", [P, WPC], f32, kind="ExternalInput")
    u_d = nc.dram_tensor("u", [P, WPC * DF], f32, kind="ExternalInput")
    w1_d = nc.dram_tensor("w1", [DF, 500], f32, kind="ExternalInput")
    b1_d = nc.dram_tensor("b1", [125, 4], f32, kind="ExternalInput")
    w23_d = nc.dram_tensor("w23", [125, 4], f32, kind="ExternalInput")
    z_d = nc.dram_tensor("z", [1, ZCOLS], f32, kind="ExternalOutput")

    Relu = mybir.ActivationFunctionType.Relu
    with tile.TileContext(nc) as tc:
        with tc.tile_pool(name="cst", bufs=1) as cst, \
             tc.tile_pool(name="g", bufs=3) as gpl, \
             tc.tile_pool(name="oh", bufs=2) as ohp, \
             tc.tile_pool(name="wk", bufs=2) as wk, \
             tc.tile_pool(name="ps", bufs=2, space="PSUM") as ps:
            ilo_t = cst.tile([128, WPC * nlo // 16], mybir.dt.int16)
            nc.sync.dma_start(out=ilo_t[:], in_=ilo_d[:])
            ihi_t = cst.tile([128, WPC * nhi // 16], mybir.dt.int16)
            nc.sync.dma_start(out=ihi_t[:], in_=ihi_d[:])
            dloc_t = cst.tile([P, WPC * t_w], f32)
            nc.sync.dma_start(out=dloc_t[:], in_=dloc_d[:])
            iota_t = cst.tile([P, P], f32)
            nc.sync.dma_start(out=iota_t[:], in_=iota_d[:])
            ident_t = cst.tile([P, P], f32)
            nc.sync.dma_start(out=ident_t[:], in_=ident_d[:])
            disw_t = cst.tile([P, WPC], f32)
            nc.sync.dma_start(out=disw_t[:], in_=disw_d[:])
            u_t = cst.tile([P, WPC, DF], f32)
            nc.sync.dma_start(out=u_t[:], in_=u_d[:].rearrange("p (w f) -> p w f", w=WPC))
            w1_t = cst.tile([DF, 500], f32)
            nc.sync.dma_start(out=w1_t[:], in_=w1_d[:])
            b1_t = cst.tile([125, 4], f32)
            nc.sync.dma_start(out=b1_t[:], in_=b1_d[:])
            w23_t = cst.tile([125, 4], f32)
            nc.sync.dma_start(out=w23_t[:], in_=w23_d[:])
            z_sb = cst.tile([1, ZCOLS], f32)

            for _rep in range(reps):
                for grp in range(NGRP):
                    wins = list(range(grp * GRP, min((grp + 1) * GRP, WPC)))
                    gt_ps = ps.tile([DF, GRP * P], f32, space="PSUM", tag="gtps")
                    for wi, w in enumerate(wins):
                        glo = gpl.tile([P, t_lo, D], f32, tag="glo")
                        nc.gpsimd.dma_gather(
                            out_ap=glo[:], in_ap=xpad_d[:GAP + 1, :],
                            idxs_ap=ilo_t[:, w * nlo // 16:(w + 1) * nlo // 16],
                            num_idxs=nlo, num_idxs_reg=nlo, elem_size=D,
                            single_packet=False)
                        ghi = gpl.tile([P, t_hi, D], f32, tag="ghi")
                        nc.gpsimd.dma_gather(
                            out_ap=ghi[:], in_ap=xpad_d[GAP + 1:, :],
                            idxs_ap=ihi_t[:, w * nhi // 16:(w + 1) * nhi // 16],
                            num_idxs=nhi, num_idxs_reg=nhi, elem_size=D,
                            single_packet=False)
                        slo = wk.tile([P, t_lo, DF], f32, tag="slo")
                        nc.vector.tensor_tensor(
                            out=slo[:], in0=glo[:, :, :DF],
                            in1=glo[:, :, DF:DF + 1].to_broadcast([P, t_lo, DF]),
                            op=mybir.AluOpType.mult)
                        shi = wk.tile([P, t_hi, DF], f32, tag="shi")
                        nc.vector.tensor_tensor(
                            out=shi[:], in0=ghi[:, :, :DF],
                            in1=ghi[:, :, DF:DF + 1].to_broadcast([P, t_hi, DF]),
                            op=mybir.AluOpType.mult)
                        oh = ohp.tile([P, t_w, P], f32, tag="oh")
                        nc.vector.tensor_tensor(
                            out=oh[:],
                            in0=dloc_t[:, w * t_w:(w + 1) * t_w].unsqueeze(2).to_broadcast([P, t_w, P]),
                            in1=iota_t[:].unsqueeze(1).to_broadcast([P, t_w, P]),
                            op=mybir.AluOpType.is_equal)
                        seg = ps.tile([P, DF], f32, space="PSUM", tag="seg")
                        for t in range(t_w):
                            rhs = slo[:, t, :] if t < t_lo else shi[:, t - t_lo, :]
                            nc.tensor.matmul(out=seg[:], lhsT=oh[:, t, :], rhs=rhs,
                                             start=(t == 0), stop=(t == t_w - 1))
                        g_sb = wk.tile([P, DF], f32, tag="gsb")
                        nc.vector.tensor_tensor(out=g_sb[:], in0=seg[:],
                                                in1=u_t[:, w, :],
                                                op=mybir.AluOpType.add)
                        nc.vector.tensor_tensor(
                            out=g_sb[:], in0=g_sb[:],
                            in1=disw_t[:, w:w + 1].to_broadcast([P, DF]),
                            op=mybir.AluOpType.mult)
                        nc.tensor.transpose(out=gt_ps[:, wi * P:(wi + 1) * P],
                                            in_=g_sb[:], identity=ident_t[:])
                    gt_sb = wk.tile([DF, GRP * P], f32, tag="gtsb")
                    nc.vector.tensor_copy(out=gt_sb[:], in_=gt_ps[:])
                    zrow = ps.tile([1, GRP * P], f32, space="PSUM", tag="zrow")
                    for c in range(4):
                        o1 = ps.tile([125, GRP * P], f32, space="PSUM", tag="o1")
                        nc.tensor.matmul(out=o1[:], lhsT=w1_t[:, c * 125:(c + 1) * 125],
                                         rhs=gt_sb[:], start=True, stop=True)
                        x1 = wk.tile([125, GRP * P], f32, tag="x1")
                        nc.scalar.activation(x1[:], o1[:], Relu,
                                             bias=b1_t[:, c:c + 1], scale=1.0)
                        nc.tensor.matmul(out=zrow[:], lhsT=w23_t[:, c:c + 1],
                                         rhs=x1[:], start=(c == 0), stop=(c == 3))
                    nc.vector.tensor_copy(
                        out=z_sb[:, grp * GRP * P:(grp + 1) * GRP * P], in_=zrow[:])
            nc.sync.dma_start(out=z_d[:], in_=z_sb[:])
    nc.compile()
    return nc


def build_pass2(t_lo, t_hi, reps=1):
    nlo, nhi = t_lo * P, t_hi * P
    t_w = t_lo + t_hi
    nc = bacc.Bacc("TRN2", target_bir_lowering=False, debug=False,
                   num_devices=NCORES)
    f32 = mybir.dt.float32
    zpad_d = nc.dram_tensor("zpad", [NROWS, D], f32, kind="ExternalInput")
    ilo_d = nc.dram_tensor("ilo", [128, WPC * nlo // 16], mybir.dt.int16, kind="ExternalInput")
    ihi_d = nc.dram_tensor("ihi", [128, WPC * nhi // 16], mybir.dt.int16, kind="ExternalInput")
    dloc_d = nc.dram_tensor("dloc", [P, WPC * t_w], f32, kind="ExternalInput")
    iota_d = nc.dram_tensor("iota", [P, P], f32, kind="ExternalInput")
    disw_d = nc.dram_tensor("disw", [P, WPC], f32, kind="ExternalInput")
    zpw_d = nc.dram_tensor("zpw", [P, WPC], f32, kind="ExternalInput")
    y_d = nc.dram_tensor("y", [P, WPC], f32, kind="ExternalOutput")

    with tile.TileContext(nc) as tc:
        with tc.tile_pool(name="cst", bufs=1) as cst, \
             tc.tile_pool(name="g", bufs=3) as gpl, \
             tc.tile_pool(name="oh", bufs=2) as ohp, \
             tc.tile_pool(name="ps", bufs=4, space="PSUM") as ps:
            ilo_t = cst.tile([128, WPC * nlo // 16], mybir.dt.int16)
            nc.sync.dma_start(out=ilo_t[:], in_=ilo_d[:])
            ihi_t = cst.tile([128, WPC * nhi // 16], mybir.dt.int16)
            nc.sync.dma_start(out=ihi_t[:], in_=ihi_d[:])
            dloc_t = cst.tile([P, WPC * t_w], f32)
            nc.sync.dma_start(out=dloc_t[:], in_=dloc_d[:])
            iota_t = cst.tile([P, P], f32)
            nc.sync.dma_start(out=iota_t[:], in_=iota_d[:])
            disw_t = cst.tile([P, WPC], f32)
            nc.sync.dma_start(out=disw_t[:], in_=disw_d[:])
            zpw_t = cst.tile([P, WPC], f32)
            nc.sync.dma_start(out=zpw_t[:], in_=zpw_d[:])
            yacc = cst.tile([P, WPC], f32)

            for _rep in range(reps):
                for w in range(WPC):
                    glo = gpl.tile([P, t_lo, D], f32, tag="glo")
                    nc.gpsimd.dma_gather(
                        out_ap=glo[:], in_ap=zpad_d[:GAP + 1, :],
                        idxs_ap=ilo_t[:, w * nlo // 16:(w + 1) * nlo // 16],
                        num_idxs=nlo, num_idxs_reg=nlo, elem_size=D,
                        single_packet=False)
                    ghi = gpl.tile([P, t_hi, D], f32, tag="ghi")
                    nc.gpsimd.dma_gather(
                        out_ap=ghi[:], in_ap=zpad_d[GAP + 1:, :],
                        idxs_ap=ihi_t[:, w * nhi // 16:(w + 1) * nhi // 16],
                        num_idxs=nhi, num_idxs_reg=nhi, elem_size=D,
                        single_packet=False)
                    oh = ohp.tile([P, t_w, P], f32, tag="oh")
                    nc.vector.tensor_tensor(
                        out=oh[:],
                        in0=dloc_t[:, w * t_w:(w + 1) * t_w].unsqueeze(2).to_broadcast([P, t_w, P]),
                        in1=iota_t[:].unsqueeze(1).to_broadcast([P, t_w, P]),
                        op=mybir.AluOpType.is_equal)
                    seg = ps.tile([P, 1], f32, space="PSUM", tag="seg")
                    for t in range(t_w):
                        rhs = glo[:, t, 0:1] if t < t_lo else ghi[:, t - t_lo, 0:1]
                        nc.tensor.matmul(out=seg[:], lhsT=oh[:, t, :], rhs=rhs,
                                         start=(t == 0), stop=(t == t_w - 1))
                    nc.vector.tensor_copy(out=yacc[:, w:w + 1], in_=seg[:])
            yout = cst.tile([P, WPC], f32)
            nc.vector.tensor_tensor(out=yout[:], in0=yacc[:], in1=zpw_t[:],
                                    op=mybir.AluOpType.add)
            nc.vector.tensor_tensor(out=yout[:], in0=yout[:], in1=disw_t[:],
                                    op=mybir.AluOpType.mult)
            nc.sync.dma_start(out=y_d[:], in_=yout[:])
    nc.compile()
    return nc


def pass1_inmaps(pp, W1, b1, w23):
    maps = []
    for k in range(NCORES):
        c = pp["per_core"][k]
        maps.append({
            "xpad": pp["Xpad"],
            "ilo": c["ilo"], "ihi": c["ihi"], "dloc": c["dloc"],
            "iota": IOTA, "ident": IDENT,
            "disw": c["disw"],
            "u": c["U"].reshape(P, WPC * DF),
            "w1": np.ascontiguousarray(W1, dtype=np.float32),
            "b1": np.asarray(b1, dtype=np.float32).reshape(4, 125).T.copy(),
            "w23": np.asarray(w23, dtype=np.float32).reshape(4, 125).T.copy(),
        })
    return maps


def pass2_inmaps(pp, z_lbl):
    zp_lbl = pp["dis_lbl"] * z_lbl
    Zpad = np.zeros((NROWS, D), dtype=np.float32)
    Zpad[_row_of(np.arange(NPAD)), 0] = zp_lbl
    maps = []
    for k in range(NCORES):
        c = pp["per_core"][k]
        lblk = np.arange(WPC * P) + WPC * k * P
        zpw = zp_lbl[lblk].reshape(WPC, P).T.copy()
        maps.append({
            "zpad": Zpad,
            "ilo": c["ilo"], "ihi": c["ihi"], "dloc": c["dloc"],
            "iota": IOTA, "disw": c["disw"], "zpw": zpw,
        })
    return maps


def kernel(state, edge_attr, edge_index, W1, b1, W2, b2, W3, b3):
    state = np.asarray(state)
    edge_attr = np.asarray(edge_attr)
    edge_index = np.asarray(edge_index)
    pp = _preprocess(state, edge_attr, edge_index)
    w23 = np.asarray(W2, dtype=np.float32) @ np.asarray(W3, dtype=np.float32)
    c2 = float((np.asarray(b2, dtype=np.float32) @ np.asarray(W3, dtype=np.float32)
                + np.asarray(b3, dtype=np.float32))[0])

    nc1 = build_pass1(pp["t_lo"], pp["t_hi"])
    r1 = run_bass_kernel_spmd(nc1, pass1_inmaps(pp, W1, b1, w23),
                              core_ids=list(range(NCORES)))
    z_lbl = np.zeros(NPAD, dtype=np.float32)
    for k in range(NCORES):
        z_lbl[WPC * k * P:WPC * (k + 1) * P] = r1.results[k]["z"][0][:WPC * P]

    nc2 = build_pass2(pp["t_lo"], pp["t_hi"])
    r2 = run_bass_kernel_spmd(nc2, pass2_inmaps(pp, z_lbl),
                              core_ids=list(range(NCORES)))
    y_lbl = np.zeros(NPAD, dtype=np.float32)
    for k in range(NCORES):
        y_lbl[WPC * k * P:WPC * (k + 1) * P] = r2.results[k]["y"].T.reshape(-1)
    return (y_lbl[pp["sigma"]] + c2)[:, None].astype(np.float32)


# revision 2
# speedup vs baseline: 3.4725x; 3.4725x over previous
"""Trainium2 Bass kernel for nn_CriticGCN (2-layer GCN critic, 50000 nodes,
800000 edges, 8 NeuronCores).

Algebraic reformulation (exact):
  A = S_dT diag(dis_s*dis_d) S_s + diag(dis^2)   (GCN norm adjacency)
  layer1: out1 = A @ (X W1) + b1 = ((A @ X) W1) + b1        (associativity)
  x1 = relu(out1); y = (A @ (x1 W2) + b2) @ W3 + b3
     = A @ (x1 (W2 W3)) + (b2 W3 + b3)                       (linearity)
  and A @ v = dis * (segsum_dst(dis_src * v[src]) + dis * v) per column.

So the device does two sparse passes (20-dim then 1-dim features) plus a
small dense chain; W2@W3 collapses layer 2's feature dim to 1.

Sharding: nodes are relabeled by a balance permutation and dst-sharded
across 8 cores (49 windows of 128 labels each). Edges are grouped per
(window, src<32768) and padded to fixed tile counts; per-edge rows are
fetched with dma_gather (256B rows from an HBM table), segment-summed via
one-hot matmuls on the TensorEngine accumulating in PSUM.
"""
import numpy as np
import concourse.bacc as bacc
import concourse.mybir as mybir
import concourse.tile as tile
from concourse.bass_utils import run_bass_kernel_spmd

P = 128
NCORES = 8
WPC = 49
NWIN = NCORES * WPC
NPAD = NWIN * P
GAP = 32767
NROWS = NPAD + 2
D = 64
DF = 20
GRP = 4
NGRP = (WPC + GRP - 1) // GRP
ZCOLS = NGRP * GRP * P

IOTA = np.broadcast_to(np.arange(P, dtype=np.float32)[None, :], (P, P)).copy()
IDENT = np.eye(P, dtype=np.float32)


def _row_of(lbl):
    return np.where(lbl < GAP, lbl, lbl + 1)


def _preprocess(state, edge_attr, edge_index):
    X = np.concatenate([state.reshape(-1, edge_attr.shape[1]),
                        edge_attr], 0).astype(np.float32)
    n = X.shape[0]
    src = edge_index[0].astype(np.int64)
    dst = edge_index[1].astype(np.int64)

    deg = np.bincount(dst, minlength=n) + 1
    dis = (1.0 / np.sqrt(deg)).astype(np.float32)

    order = np.argsort(-deg, kind="stable")
    sigma = np.empty(n, dtype=np.int64)
    sigma[order] = (np.arange(n) % NWIN) * P + np.arange(n) // NWIN

    s_row = _row_of(sigma[src])
    d_lbl = sigma[dst]
    d_win = d_lbl // P
    d_loc = d_lbl % P
    is_hi = s_row >= GAP + 1

    key = d_win * 2 + is_hi
    eorder = np.argsort(key, kind="stable")
    s_row_s = s_row[eorder]
    d_loc_s = d_loc[eorder]
    counts = np.bincount(key[eorder], minlength=NWIN * 2)
    off = np.concatenate([[0], np.cumsum(counts)])
    t_lo = int(np.ceil(counts[0::2].max() / P))
    t_hi = int(np.ceil(counts[1::2].max() / P))
    nlo, nhi = t_lo * P, t_hi * P

    idx_lo = np.full((NWIN, nlo), GAP, dtype=np.int64)
    idx_hi = np.full((NWIN, nhi), NPAD + 1 - (GAP + 1), dtype=np.int64)
    dloc = np.zeros((NWIN, (t_lo + t_hi) * P), dtype=np.int64)
    for w in range(NWIN):
        lo0, lo1 = off[2 * w], off[2 * w + 1]
        hi0, hi1 = off[2 * w + 1], off[2 * w + 2]
        klo, khi = lo1 - lo0, hi1 - hi0
        idx_lo[w, :klo] = s_row_s[lo0:lo1]
        idx_hi[w, :khi] = s_row_s[hi0:hi1] - (GAP + 1)
        dloc[w, :klo] = d_loc_s[lo0:lo1]
        dloc[w, nlo:nlo + khi] = d_loc_s[hi0:hi1]

    Xpad = np.zeros((NROWS, D), dtype=np.float32)
    rows = _row_of(sigma)
    Xpad[rows, :DF] = X
    Xpad[rows, DF] = dis

    dis_lbl = np.zeros(NPAD, dtype=np.float32)
    dis_lbl[sigma] = dis
    U_lbl = np.zeros((NPAD, DF), dtype=np.float32)
    U_lbl[sigma] = X * dis[:, None]

    def wrap16(a):
        return np.tile(a.astype(np.int16).reshape(-1, 16).T, (8, 1))

    per_core = []
    for k in range(NCORES):
        wr = range(WPC * k, WPC * (k + 1))
        ilo = np.concatenate([wrap16(idx_lo[w]) for w in wr], axis=1)
        ihi = np.concatenate([wrap16(idx_hi[w]) for w in wr], axis=1)
        dl = np.concatenate(
            [dloc[w].reshape(t_lo + t_hi, P).T.astype(np.float32) for w in wr],
            axis=1)
        lblk = np.arange(WPC * P) + WPC * k * P
        disw = dis_lbl[lblk].reshape(WPC, P).T.copy()
        Uw = U_lbl[lblk].reshape(WPC, P, DF).transpose(1, 0, 2).copy()
        per_core.append(dict(ilo=ilo, ihi=ihi, dloc=dl, disw=disw, U=Uw))
    return dict(per_core=per_core, Xpad=Xpad, sigma=sigma, dis_lbl=dis_lbl,
                t_lo=t_lo, t_hi=t_hi)


def build_pass1(t_lo, t_hi, reps=1):
    nlo, nhi = t_lo * P, t_hi * P
    t_w = t_lo + t_hi
    nc = bacc.Bacc("TRN2", target_bir_lowering=False, debug=False,
                   num_devices=NCORES)
    f32 = mybir.dt.float32
    xpad_d = nc.dram_tensor("xpad", [NROWS, D], f32, kind="ExternalInput")
    ilo_d = nc.dram_tensor("ilo", [128, WPC * nlo // 16], mybir.dt.int16, kind="ExternalInput")
    ihi_d = nc.dram_tensor("ihi", [128, WPC * nhi // 16], mybir.dt.int16, kind="ExternalInput")
    dloc_d = nc.dram_tensor("dloc", [P, WPC * t_w], f32, kind="ExternalInput")
    iota_d = nc.dram_tensor("iota", [P, P], f32, kind="ExternalInput")
    ident_d = nc.dram_tensor("ident", [P, P], f32, kind="ExternalInput")
    disw_d = nc.dram_tensor("disw", [P, WPC], f32, kind="ExternalInput")
    u_d = nc.dram_tensor("u", [P, WPC * DF], f32, kind="ExternalInput")
    w1_d = nc.dram_tensor("w1", [DF, 500], f32, kind="ExternalInput")
    b1_d = nc.dram_tensor("b1", [125, 4], f32, kind="ExternalInput")
    w23_d = nc.dram_tensor("w23", [125, 4], f32, kind="ExternalInput")
    z_d = nc.dram_tensor("z", [1, ZCOLS], f32, kind="ExternalOutput")

    Relu = mybir.ActivationFunctionType.Relu
    with tile.TileContext(nc) as tc:
        with tc.tile_pool(name="cst", bufs=1) as cst, \
             tc.tile_pool(name="g", bufs=3) as gpl, \
             tc.tile_pool(name="oh", bufs=2) as ohp, \
             tc.tile_pool(name="wk", bufs=2) as wk, \
             tc.tile_pool(name="ps", bufs=2, space="PSUM") as ps:
            ilo_t = cst.tile([128, WPC * nlo // 16], mybir.dt.int16)
            nc.sync.dma_start(out=ilo_t[:], in_=ilo_d[:])
            ihi_t = cst.tile([128, WPC * nhi // 16], mybir.dt.int16)
            nc.sync.dma_start(out=ihi_t[:], in_=ihi_d[:])
            dloc_t = cst.tile([P, WPC * t_w], f32)
            nc.sync.dma_start(out=dloc_t[:], in_=dloc_d[:])
            iota_t = cst.tile([P, P], f32)
            nc.sync.dma_start(out=iota_t[:], in_=iota_d[:])
            ident_t = cst.tile([P, P], f32)
            nc.sync.dma_start(out=ident_t[:], in_=ident_d[:])
            disw_t = cst.tile([P, WPC], f32)
            nc.sync.dma_start(out=disw_t[:], in_=disw_d[:])
            u_t = cst.tile([P, WPC, DF], f32)
            nc.sync.dma_start(out=u_t[:], in_=u_d[:].rearrange("p (w f) -> p w f", w=WPC))
            w1_t = cst.tile([DF, 500], f32)
            nc.sync.dma_start(out=w1_t[:], in_=w1_d[:])
            b1_t = cst.tile([125, 4], f32)
            nc.sync.dma_start(out=b1_t[:], in_=b1_d[:])
            w23_t = cst.tile([125, 4], f32)
            nc.sync.dma_start(out=w23_t[:], in_=w23_d[:])
            z_sb = cst.tile([1, ZCOLS], f32)

            for _rep in range(reps):
                for grp in range(NGRP):
                    wins = list(range(grp * GRP, min((grp + 1) * GRP, WPC)))
                    gt_ps = ps.tile([DF, GRP * P], f32, space="PSUM", tag="gtps")
                    for wi, w in enumerate(wins):
                        glo = gpl.tile([P, t_lo, D], f32, tag="glo")
                        nc.gpsimd.dma_gather(
                            out_ap=glo[:], in_ap=xpad_d[:GAP + 1, :],
                            idxs_ap=ilo_t[:, w * nlo // 16:(w + 1) * nlo // 16],
                            num_idxs=nlo, num_idxs_reg=nlo, elem_size=D,
                            single_packet=False)
                        ghi = gpl.tile([P, t_hi, D], f32, tag="ghi")
                        nc.gpsimd.dma_gather(
                            out_ap=ghi[:], in_ap=xpad_d[GAP + 1:, :],
                            idxs_ap=ihi_t[:, w * nhi // 16:(w + 1) * nhi // 16],
                            num_idxs=nhi, num_idxs_reg=nhi, elem_size=D,
                            single_packet=False)
                        slo = wk.tile([P, t_lo, DF], f32, tag="slo")
                        nc.vector.tensor_tensor(
                            out=slo[:], in0=glo[:, :, :DF],
                            in1=glo[:, :, DF:DF + 1].to_broadcast([P, t_lo, DF]),
                            op=mybir.AluOpType.mult)
                        shi = wk.tile([P, t_hi, DF], f32, tag="shi")
                        nc.vector.tensor_tensor(
                            out=shi[:], in0=ghi[:, :, :DF],
                            in1=ghi[:, :, DF:DF + 1].to_broadcast([P, t_hi, DF]),
                            op=mybir.AluOpType.mult)
                        oh = ohp.tile([P, t_w, P], f32, tag="oh")
                        nc.vector.tensor_tensor(
                            out=oh[:],
                            in0=dloc_t[:, w * t_w:(w + 1) * t_w].unsqueeze(2).to_broadcast([P, t_w, P]),
                            in1=iota_t[:].unsqueeze(1).to_broadcast([P, t_w, P]),
                            op=mybir.AluOpType.is_equal)
                        seg = ps.tile([P, DF], f32, space="PSUM", tag="seg")
                        for t in range(t_w):
                            rhs = slo[:, t, :] if t < t_lo else shi[:, t - t_lo, :]
                            nc.tensor.matmul(out=seg[:], lhsT=oh[:, t, :], rhs=rhs,
                                             start=(t == 0), stop=(t == t_w - 1))
                        g_sb = wk.tile([P, DF], f32, tag="gsb")
                        nc.vector.tensor_tensor(out=g_sb[:], in0=seg[:],
                                                in1=u_t[:, w, :],
                                                op=mybir.AluOpType.add)
                        nc.vector.tensor_tensor(
                            out=g_sb[:], in0=g_sb[:],
                            in1=disw_t[:, w:w + 1].to_broadcast([P, DF]),
                            op=mybir.AluOpType.mult)
                        nc.tensor.transpose(out=gt_ps[:, wi * P:(wi + 1) * P],
                                            in_=g_sb[:], identity=ident_t[:])
                    gt_sb = wk.tile([DF, GRP * P], f32, tag="gtsb")
                    nc.vector.tensor_copy(out=gt_sb[:], in_=gt_ps[:])
                    zrow = ps.tile([1, GRP * P], f32, space="PSUM", tag="zrow")
                    for c in range(4):
                        o1 = ps.tile([125, GRP * P], f32, space="PSUM", tag="o1")
                        nc.tensor.matmul(out=o1[:], lhsT=w1_t[:, c * 125:(c + 1) * 125],
                                         rhs=gt_sb[:], start=True, stop=True)
                        x1 = wk.tile([125, GRP * P], f32, tag="x1")
                        nc.scalar.activation(x1[:], o1[:], Relu,
                                             bias=b1_t[:, c:c + 1], scale=1.0)
                        nc.tensor.matmul(out=zrow[:], lhsT=w23_t[:, c:c + 1],
                                         rhs=x1[:], start=(c == 0), stop=(c == 3))
                    nc.vector.tensor_copy(
                        out=z_sb[:, grp * GRP * P:(grp + 1) * GRP * P], in_=zrow[:])
            nc.sync.dma_start(out=z_d[:], in_=z_sb[:])
    nc.compile()
    return nc


def build_pass2(t_lo, t_hi, reps=1):
    nlo, nhi = t_lo * P, t_hi * P
    t_w = t_lo + t_hi
    nc = bacc.Bacc("TRN2", target_bir_lowering=False, debug=False,
                   num_devices=NCORES)
    f32 = mybir.dt.float32
    zpad_d = nc.dram_tensor("zpad", [NROWS, D], f32, kind="ExternalInput")
    ilo_d = nc.dram_tensor("ilo", [128, WPC * nlo // 16], mybir.dt.int16, kind="ExternalInput")
    ihi_d = nc.dram_tensor("ihi", [128, WPC * nhi // 16], mybir.dt.int16, kind="ExternalInput")
    dloc_d = nc.dram_tensor("dloc", [P, WPC * t_w], f32, kind="ExternalInput")
    iota_d = nc.dram_tensor("iota", [P, P], f32, kind="ExternalInput")
    disw_d = nc.dram_tensor("disw", [P, WPC], f32, kind="ExternalInput")
    zpw_d = nc.dram_tensor("zpw", [P, WPC], f32, kind="ExternalInput")
    y_d = nc.dram_tensor("y", [P, WPC], f32, kind="ExternalOutput")

    with tile.TileContext(nc) as tc:
        with tc.tile_pool(name="cst", bufs=1) as cst, \
             tc.tile_pool(name="g", bufs=3) as gpl, \
             tc.tile_pool(name="oh", bufs=2) as ohp, \
             tc.tile_pool(name="ps", bufs=4, space="PSUM") as ps:
            ilo_t = cst.tile([128, WPC * nlo // 16], mybir.dt.int16)
            nc.sync.dma_start(out=ilo_t[:], in_=ilo_d[:])
            ihi_t = cst.tile([128, WPC * nhi // 16], mybir.dt.int16)
            nc.sync.dma_start(out=ihi_t[:], in_=ihi_d[:])
            dloc_t = cst.tile([P, WPC * t_w], f32)
            nc.sync.dma_start(out=dloc_t[:], in_=dloc_d[:])
            iota_t = cst.tile([P, P], f32)
            nc.sync.dma_start(out=iota_t[:], in_=iota_d[:])
            disw_t = cst.tile([P, WPC], f32)
            nc.sync.dma_start(out=disw_t[:], in_=disw_d[:])
            zpw_t = cst.tile([P, WPC], f32)
            nc.sync.dma_start(out=zpw_t[:], in_=zpw_d[:])
            yacc = cst.tile([P, WPC], f32)

            for _rep in range(reps):
                for w in range(WPC):
                    glo = gpl.tile([P, t_lo, D], f32, tag="glo")
                    nc.gpsimd.dma_gather(
                        out_ap=glo[:], in_ap=zpad_d[:GAP + 1, :],
                        idxs_ap=ilo_t[:, w * nlo // 16:(w + 1) * nlo // 16],
                        num_idxs=nlo, num_idxs_reg=nlo, elem_size=D,
                        single_packet=False)
                    ghi = gpl.tile([P, t_hi, D], f32, tag="ghi")
                    nc.gpsimd.dma_gather(
                        out_ap=ghi[:], in_ap=zpad_d[GAP + 1:, :],
                        idxs_ap=ihi_t[:, w * nhi // 16:(w + 1) * nhi // 16],
                        num_idxs=nhi, num_idxs_reg=nhi, elem_size=D,
                        single_packet=False)
                    oh = ohp.tile([P, t_w, P], f32, tag="oh")
                    nc.vector.tensor_tensor(
                        out=oh[:],
                        in0=dloc_t[:, w * t_w:(w + 1) * t_w].unsqueeze(2).to_broadcast([P, t_w, P]),
                        in1=iota_t[:].unsqueeze(1).to_broadcast([P, t_w, P]),
                        op=mybir.AluOpType.is_equal)
                    # rhs is 20 cols wide (cols 1:20 of zpad rows are zero) —
                    # N=1 matmuls hit a severe per-instruction floor on HW.
                    seg = ps.tile([P, DF], f32, space="PSUM", tag="seg")
                    for t in range(t_w):
                        rhs = glo[:, t, :DF] if t < t_lo else ghi[:, t - t_lo, :DF]
                        nc.tensor.matmul(out=seg[:], lhsT=oh[:, t, :], rhs=rhs,
                                         start=(t == 0), stop=(t == t_w - 1))
                    nc.vector.tensor_copy(out=yacc[:, w:w + 1], in_=seg[:, 0:1])
            yout = cst.tile([P, WPC], f32)
            nc.vector.tensor_tensor(out=yout[:], in0=yacc[:], in1=zpw_t[:],
                                    op=mybir.AluOpType.add)
            nc.vector.tensor_tensor(out=yout[:], in0=yout[:], in1=disw_t[:],
                                    op=mybir.AluOpType.mult)
            nc.sync.dma_start(out=y_d[:], in_=yout[:])
    nc.compile()
    return nc


def pass1_inmaps(pp, W1, b1, w23):
    maps = []
    for k in range(NCORES):
        c = pp["per_core"][k]
        maps.append({
            "xpad": pp["Xpad"],
            "ilo": c["ilo"], "ihi": c["ihi"], "dloc": c["dloc"],
            "iota": IOTA, "ident": IDENT,
            "disw": c["disw"],
            "u": c["U"].reshape(P, WPC * DF),
            "w1": np.ascontiguousarray(W1, dtype=np.float32),
            "b1": np.asarray(b1, dtype=np.float32).reshape(4, 125).T.copy(),
            "w23": np.asarray(w23, dtype=np.float32).reshape(4, 125).T.copy(),
        })
    return maps


def pass2_inmaps(pp, z_lbl):
    zp_lbl = pp["dis_lbl"] * z_lbl
    Zpad = np.zeros((NROWS, D), dtype=np.float32)
    Zpad[_row_of(np.arange(NPAD)), 0] = zp_lbl
    maps = []
    for k in range(NCORES):
        c = pp["per_core"][k]
        lblk = np.arange(WPC * P) + WPC * k * P
        zpw = zp_lbl[lblk].reshape(WPC, P).T.copy()
        maps.append({
            "zpad": Zpad,
            "ilo": c["ilo"], "ihi": c["ihi"], "dloc": c["dloc"],
            "iota": IOTA, "disw": c["disw"], "zpw": zpw,
        })
    return maps


def kernel(state, edge_attr, edge_index, W1, b1, W2, b2, W3, b3):
    state = np.asarray(state)
    edge_attr = np.asarray(edge_attr)
    edge_index = np.asarray(edge_index)
    pp = _preprocess(state, edge_attr, edge_index)
    w23 = np.asarray(W2, dtype=np.float32) @ np.asarray(W3, dtype=np.float32)
    c2 = float((np.asarray(b2, dtype=np.float32) @ np.asarray(W3, dtype=np.float32)
                + np.asarray(b3, dtype=np.float32))[0])

    nc1 = build_pass1(pp["t_lo"], pp["t_hi"])
    r1 = run_bass_kernel_spmd(nc1, pass1_inmaps(pp, W1, b1, w23),
                              core_ids=list(range(NCORES)))
    z_lbl = np.zeros(NPAD, dtype=np.float32)
    for k in range(NCORES):
        z_lbl[WPC * k * P:WPC * (k + 1) * P] = r1.results[k]["z"][0][:WPC * P]

    nc2 = build_pass2(pp["t_lo"], pp["t_hi"])
    r2 = run_bass_kernel_spmd(nc2, pass2_inmaps(pp, z_lbl),
                              core_ids=list(range(NCORES)))
    y_lbl = np.zeros(NPAD, dtype=np.float32)
    for k in range(NCORES):
        y_lbl[WPC * k * P:WPC * (k + 1) * P] = r2.results[k]["y"].T.reshape(-1)
    return (y_lbl[pp["sigma"]] + c2)[:, None].astype(np.float32)


# revision 3
# speedup vs baseline: 3.7182x; 1.0708x over previous
"""Trainium2 Bass kernel for nn_CriticGCN (2-layer GCN critic, 50000 nodes,
800000 edges, 8 NeuronCores).

Algebraic reformulation (exact):
  A = S_dT diag(dis_s*dis_d) S_s + diag(dis^2)   (GCN norm adjacency)
  layer1: out1 = A @ (X W1) + b1 = ((A @ X) W1) + b1        (associativity)
  x1 = relu(out1); y = (A @ (x1 W2) + b2) @ W3 + b3
     = A @ (x1 (W2 W3)) + (b2 W3 + b3)                       (linearity)
  and A @ v = dis * (segsum_dst(dis_src * v[src]) + dis * v) per column.

So the device does two sparse passes (20-dim then 1-dim features) plus a
small dense chain; W2@W3 collapses layer 2's feature dim to 1.

Sharding: nodes are relabeled by a balance permutation and dst-sharded
across 8 cores (49 windows of 128 labels each). Edges are grouped per
(window, src<32768) and padded to fixed tile counts; per-edge rows are
fetched with dma_gather (256B rows from an HBM table), segment-summed via
one-hot matmuls on the TensorEngine accumulating in PSUM.
"""
import numpy as np
import concourse.bacc as bacc
import concourse.mybir as mybir
import concourse.tile as tile
from concourse.bass_utils import run_bass_kernel_spmd

P = 128
NCORES = 8
WPC = 49
NWIN = NCORES * WPC
NPAD = NWIN * P
GAP = 32767
NROWS = NPAD + 2
D = 64
DF = 20
GRP = 4
NGRP = (WPC + GRP - 1) // GRP
ZCOLS = NGRP * GRP * P

IOTA = np.broadcast_to(np.arange(P, dtype=np.float32)[None, :], (P, P)).copy()
IDENT = np.eye(P, dtype=np.float32)


def _row_of(lbl):
    return np.where(lbl < GAP, lbl, lbl + 1)


def _preprocess(state, edge_attr, edge_index):
    X = np.concatenate([state.reshape(-1, edge_attr.shape[1]),
                        edge_attr], 0).astype(np.float32)
    n = X.shape[0]
    src = edge_index[0].astype(np.int64)
    dst = edge_index[1].astype(np.int64)

    deg = np.bincount(dst, minlength=n) + 1
    dis = (1.0 / np.sqrt(deg)).astype(np.float32)

    order = np.argsort(-deg, kind="stable")
    sigma = np.empty(n, dtype=np.int64)
    sigma[order] = (np.arange(n) % NWIN) * P + np.arange(n) // NWIN

    s_row = _row_of(sigma[src])
    d_lbl = sigma[dst]
    d_win = d_lbl // P
    d_loc = d_lbl % P
    is_hi = s_row >= GAP + 1

    key = d_win * 2 + is_hi
    eorder = np.argsort(key, kind="stable")
    s_row_s = s_row[eorder]
    d_loc_s = d_loc[eorder]
    counts = np.bincount(key[eorder], minlength=NWIN * 2)
    off = np.concatenate([[0], np.cumsum(counts)])
    t_lo = int(np.ceil(counts[0::2].max() / P))
    t_hi = int(np.ceil(counts[1::2].max() / P))
    nlo, nhi = t_lo * P, t_hi * P

    idx_lo = np.full((NWIN, nlo), GAP, dtype=np.int64)
    idx_hi = np.full((NWIN, nhi), NPAD + 1 - (GAP + 1), dtype=np.int64)
    dloc = np.zeros((NWIN, (t_lo + t_hi) * P), dtype=np.int64)
    for w in range(NWIN):
        lo0, lo1 = off[2 * w], off[2 * w + 1]
        hi0, hi1 = off[2 * w + 1], off[2 * w + 2]
        klo, khi = lo1 - lo0, hi1 - hi0
        idx_lo[w, :klo] = s_row_s[lo0:lo1]
        idx_hi[w, :khi] = s_row_s[hi0:hi1] - (GAP + 1)
        dloc[w, :klo] = d_loc_s[lo0:lo1]
        dloc[w, nlo:nlo + khi] = d_loc_s[hi0:hi1]

    Xpad = np.zeros((NROWS, D), dtype=np.float32)
    rows = _row_of(sigma)
    Xpad[rows, :DF] = X
    Xpad[rows, DF] = dis

    dis_lbl = np.zeros(NPAD, dtype=np.float32)
    dis_lbl[sigma] = dis
    U_lbl = np.zeros((NPAD, DF), dtype=np.float32)
    U_lbl[sigma] = X * dis[:, None]

    def wrap16(a):
        return np.tile(a.astype(np.int16).reshape(-1, 16).T, (8, 1))

    per_core = []
    for k in range(NCORES):
        wr = range(WPC * k, WPC * (k + 1))
        ilo = np.concatenate([wrap16(idx_lo[w]) for w in wr], axis=1)
        ihi = np.concatenate([wrap16(idx_hi[w]) for w in wr], axis=1)
        dl = np.concatenate(
            [dloc[w].reshape(t_lo + t_hi, P).T.astype(np.float32) for w in wr],
            axis=1)
        lblk = np.arange(WPC * P) + WPC * k * P
        disw = dis_lbl[lblk].reshape(WPC, P).T.copy()
        Uw = U_lbl[lblk].reshape(WPC, P, DF).transpose(1, 0, 2).copy()
        per_core.append(dict(ilo=ilo, ihi=ihi, dloc=dl, disw=disw, U=Uw))
    return dict(per_core=per_core, Xpad=Xpad, sigma=sigma, dis_lbl=dis_lbl,
                t_lo=t_lo, t_hi=t_hi)


def build_pass1(t_lo, t_hi, reps=1):
    nlo, nhi = t_lo * P, t_hi * P
    t_w = t_lo + t_hi
    nc = bacc.Bacc("TRN2", target_bir_lowering=False, debug=False,
                   num_devices=NCORES)
    f32 = mybir.dt.float32
    xpad_d = nc.dram_tensor("xpad", [NROWS, D], f32, kind="ExternalInput")
    ilo_d = nc.dram_tensor("ilo", [128, WPC * nlo // 16], mybir.dt.int16, kind="ExternalInput")
    ihi_d = nc.dram_tensor("ihi", [128, WPC * nhi // 16], mybir.dt.int16, kind="ExternalInput")
    dloc_d = nc.dram_tensor("dloc", [P, WPC * t_w], f32, kind="ExternalInput")
    iota_d = nc.dram_tensor("iota", [P, P], f32, kind="ExternalInput")
    ident_d = nc.dram_tensor("ident", [P, P], f32, kind="ExternalInput")
    disw_d = nc.dram_tensor("disw", [P, WPC], f32, kind="ExternalInput")
    u_d = nc.dram_tensor("u", [P, WPC * DF], f32, kind="ExternalInput")
    w1_d = nc.dram_tensor("w1", [DF, 500], f32, kind="ExternalInput")
    b1_d = nc.dram_tensor("b1", [125, 4], f32, kind="ExternalInput")
    w23_d = nc.dram_tensor("w23", [125, 4], f32, kind="ExternalInput")
    z_d = nc.dram_tensor("z", [1, ZCOLS], f32, kind="ExternalOutput")

    Relu = mybir.ActivationFunctionType.Relu
    with tile.TileContext(nc) as tc:
        with tc.tile_pool(name="cst", bufs=1) as cst, \
             tc.tile_pool(name="g", bufs=5) as gpl, \
             tc.tile_pool(name="oh", bufs=3) as ohp, \
             tc.tile_pool(name="wk", bufs=3) as wk, \
             tc.tile_pool(name="ps", bufs=2, space="PSUM") as ps:
            ilo_t = cst.tile([128, WPC * nlo // 16], mybir.dt.int16)
            nc.sync.dma_start(out=ilo_t[:], in_=ilo_d[:])
            ihi_t = cst.tile([128, WPC * nhi // 16], mybir.dt.int16)
            nc.sync.dma_start(out=ihi_t[:], in_=ihi_d[:])
            dloc_t = cst.tile([P, WPC * t_w], f32)
            nc.sync.dma_start(out=dloc_t[:], in_=dloc_d[:])
            iota_t = cst.tile([P, P], f32)
            nc.sync.dma_start(out=iota_t[:], in_=iota_d[:])
            ident_t = cst.tile([P, P], f32)
            nc.sync.dma_start(out=ident_t[:], in_=ident_d[:])
            disw_t = cst.tile([P, WPC], f32)
            nc.sync.dma_start(out=disw_t[:], in_=disw_d[:])
            u_t = cst.tile([P, WPC, DF], f32)
            nc.sync.dma_start(out=u_t[:], in_=u_d[:].rearrange("p (w f) -> p w f", w=WPC))
            w1_t = cst.tile([DF, 500], f32)
            nc.sync.dma_start(out=w1_t[:], in_=w1_d[:])
            b1_t = cst.tile([125, 4], f32)
            nc.sync.dma_start(out=b1_t[:], in_=b1_d[:])
            w23_t = cst.tile([125, 4], f32)
            nc.sync.dma_start(out=w23_t[:], in_=w23_d[:])
            z_sb = cst.tile([1, ZCOLS], f32)

            for _rep in range(reps):
                for grp in range(NGRP):
                    wins = list(range(grp * GRP, min((grp + 1) * GRP, WPC)))
                    gt_ps = ps.tile([DF, GRP * P], f32, space="PSUM", tag="gtps")
                    for wi, w in enumerate(wins):
                        glo = gpl.tile([P, t_lo, D], f32, tag="glo")
                        nc.gpsimd.dma_gather(
                            out_ap=glo[:], in_ap=xpad_d[:GAP + 1, :],
                            idxs_ap=ilo_t[:, w * nlo // 16:(w + 1) * nlo // 16],
                            num_idxs=nlo, num_idxs_reg=nlo, elem_size=D,
                            single_packet=False)
                        ghi = gpl.tile([P, t_hi, D], f32, tag="ghi")
                        nc.gpsimd.dma_gather(
                            out_ap=ghi[:], in_ap=xpad_d[GAP + 1:, :],
                            idxs_ap=ihi_t[:, w * nhi // 16:(w + 1) * nhi // 16],
                            num_idxs=nhi, num_idxs_reg=nhi, elem_size=D,
                            single_packet=False)
                        slo = wk.tile([P, t_lo, DF], f32, tag="slo")
                        nc.vector.tensor_tensor(
                            out=slo[:], in0=glo[:, :, :DF],
                            in1=glo[:, :, DF:DF + 1].to_broadcast([P, t_lo, DF]),
                            op=mybir.AluOpType.mult)
                        shi = wk.tile([P, t_hi, DF], f32, tag="shi")
                        nc.vector.tensor_tensor(
                            out=shi[:], in0=ghi[:, :, :DF],
                            in1=ghi[:, :, DF:DF + 1].to_broadcast([P, t_hi, DF]),
                            op=mybir.AluOpType.mult)
                        oh = ohp.tile([P, t_w, P], f32, tag="oh")
                        nc.vector.tensor_tensor(
                            out=oh[:],
                            in0=dloc_t[:, w * t_w:(w + 1) * t_w].unsqueeze(2).to_broadcast([P, t_w, P]),
                            in1=iota_t[:].unsqueeze(1).to_broadcast([P, t_w, P]),
                            op=mybir.AluOpType.is_equal)
                        seg = ps.tile([P, DF], f32, space="PSUM", tag="seg")
                        for t in range(t_w):
                            rhs = slo[:, t, :] if t < t_lo else shi[:, t - t_lo, :]
                            nc.tensor.matmul(out=seg[:], lhsT=oh[:, t, :], rhs=rhs,
                                             start=(t == 0), stop=(t == t_w - 1))
                        g_sb = wk.tile([P, DF], f32, tag="gsb")
                        nc.vector.tensor_tensor(out=g_sb[:], in0=seg[:],
                                                in1=u_t[:, w, :],
                                                op=mybir.AluOpType.add)
                        nc.vector.tensor_tensor(
                            out=g_sb[:], in0=g_sb[:],
                            in1=disw_t[:, w:w + 1].to_broadcast([P, DF]),
                            op=mybir.AluOpType.mult)
                        nc.tensor.transpose(out=gt_ps[:, wi * P:(wi + 1) * P],
                                            in_=g_sb[:], identity=ident_t[:])
                    gt_sb = wk.tile([DF, GRP * P], f32, tag="gtsb")
                    nc.vector.tensor_copy(out=gt_sb[:], in_=gt_ps[:])
                    zrow = ps.tile([1, GRP * P], f32, space="PSUM", tag="zrow")
                    for c in range(4):
                        o1 = ps.tile([125, GRP * P], f32, space="PSUM", tag="o1")
                        nc.tensor.matmul(out=o1[:], lhsT=w1_t[:, c * 125:(c + 1) * 125],
                                         rhs=gt_sb[:], start=True, stop=True)
                        x1 = wk.tile([125, GRP * P], f32, tag="x1")
                        nc.scalar.activation(x1[:], o1[:], Relu,
                                             bias=b1_t[:, c:c + 1], scale=1.0)
                        nc.tensor.matmul(out=zrow[:], lhsT=w23_t[:, c:c + 1],
                                         rhs=x1[:], start=(c == 0), stop=(c == 3))
                    nc.vector.tensor_copy(
                        out=z_sb[:, grp * GRP * P:(grp + 1) * GRP * P], in_=zrow[:])
            nc.sync.dma_start(out=z_d[:], in_=z_sb[:])
    nc.compile()
    return nc


def build_pass2(t_lo, t_hi, reps=1):
    nlo, nhi = t_lo * P, t_hi * P
    t_w = t_lo + t_hi
    nc = bacc.Bacc("TRN2", target_bir_lowering=False, debug=False,
                   num_devices=NCORES)
    f32 = mybir.dt.float32
    zpad_d = nc.dram_tensor("zpad", [NROWS, D], f32, kind="ExternalInput")
    ilo_d = nc.dram_tensor("ilo", [128, WPC * nlo // 16], mybir.dt.int16, kind="ExternalInput")
    ihi_d = nc.dram_tensor("ihi", [128, WPC * nhi // 16], mybir.dt.int16, kind="ExternalInput")
    dloc_d = nc.dram_tensor("dloc", [P, WPC * t_w], f32, kind="ExternalInput")
    iota_d = nc.dram_tensor("iota", [P, P], f32, kind="ExternalInput")
    disw_d = nc.dram_tensor("disw", [P, WPC], f32, kind="ExternalInput")
    zpw_d = nc.dram_tensor("zpw", [P, WPC], f32, kind="ExternalInput")
    y_d = nc.dram_tensor("y", [P, WPC], f32, kind="ExternalOutput")

    with tile.TileContext(nc) as tc:
        with tc.tile_pool(name="cst", bufs=1) as cst, \
             tc.tile_pool(name="g", bufs=5) as gpl, \
             tc.tile_pool(name="oh", bufs=3) as ohp, \
             tc.tile_pool(name="ps", bufs=4, space="PSUM") as ps:
            ilo_t = cst.tile([128, WPC * nlo // 16], mybir.dt.int16)
            nc.sync.dma_start(out=ilo_t[:], in_=ilo_d[:])
            ihi_t = cst.tile([128, WPC * nhi // 16], mybir.dt.int16)
            nc.sync.dma_start(out=ihi_t[:], in_=ihi_d[:])
            dloc_t = cst.tile([P, WPC * t_w], f32)
            nc.sync.dma_start(out=dloc_t[:], in_=dloc_d[:])
            iota_t = cst.tile([P, P], f32)
            nc.sync.dma_start(out=iota_t[:], in_=iota_d[:])
            disw_t = cst.tile([P, WPC], f32)
            nc.sync.dma_start(out=disw_t[:], in_=disw_d[:])
            zpw_t = cst.tile([P, WPC], f32)
            nc.sync.dma_start(out=zpw_t[:], in_=zpw_d[:])
            yacc = cst.tile([P, WPC], f32)

            for _rep in range(reps):
                for w in range(WPC):
                    glo = gpl.tile([P, t_lo, D], f32, tag="glo")
                    nc.gpsimd.dma_gather(
                        out_ap=glo[:], in_ap=zpad_d[:GAP + 1, :],
                        idxs_ap=ilo_t[:, w * nlo // 16:(w + 1) * nlo // 16],
                        num_idxs=nlo, num_idxs_reg=nlo, elem_size=D,
                        single_packet=False)
                    ghi = gpl.tile([P, t_hi, D], f32, tag="ghi")
                    nc.gpsimd.dma_gather(
                        out_ap=ghi[:], in_ap=zpad_d[GAP + 1:, :],
                        idxs_ap=ihi_t[:, w * nhi // 16:(w + 1) * nhi // 16],
                        num_idxs=nhi, num_idxs_reg=nhi, elem_size=D,
                        single_packet=False)
                    oh = ohp.tile([P, t_w, P], f32, tag="oh")
                    nc.vector.tensor_tensor(
                        out=oh[:],
                        in0=dloc_t[:, w * t_w:(w + 1) * t_w].unsqueeze(2).to_broadcast([P, t_w, P]),
                        in1=iota_t[:].unsqueeze(1).to_broadcast([P, t_w, P]),
                        op=mybir.AluOpType.is_equal)
                    # rhs is 20 cols wide (cols 1:20 of zpad rows are zero) —
                    # N=1 matmuls hit a severe per-instruction floor on HW.
                    seg = ps.tile([P, DF], f32, space="PSUM", tag="seg")
                    for t in range(t_w):
                        rhs = glo[:, t, :DF] if t < t_lo else ghi[:, t - t_lo, :DF]
                        nc.tensor.matmul(out=seg[:], lhsT=oh[:, t, :], rhs=rhs,
                                         start=(t == 0), stop=(t == t_w - 1))
                    nc.vector.tensor_copy(out=yacc[:, w:w + 1], in_=seg[:, 0:1])
            yout = cst.tile([P, WPC], f32)
            nc.vector.tensor_tensor(out=yout[:], in0=yacc[:], in1=zpw_t[:],
                                    op=mybir.AluOpType.add)
            nc.vector.tensor_tensor(out=yout[:], in0=yout[:], in1=disw_t[:],
                                    op=mybir.AluOpType.mult)
            nc.sync.dma_start(out=y_d[:], in_=yout[:])
    nc.compile()
    return nc


def pass1_inmaps(pp, W1, b1, w23):
    maps = []
    for k in range(NCORES):
        c = pp["per_core"][k]
        maps.append({
            "xpad": pp["Xpad"],
            "ilo": c["ilo"], "ihi": c["ihi"], "dloc": c["dloc"],
            "iota": IOTA, "ident": IDENT,
            "disw": c["disw"],
            "u": c["U"].reshape(P, WPC * DF),
            "w1": np.ascontiguousarray(W1, dtype=np.float32),
            "b1": np.asarray(b1, dtype=np.float32).reshape(4, 125).T.copy(),
            "w23": np.asarray(w23, dtype=np.float32).reshape(4, 125).T.copy(),
        })
    return maps


def pass2_inmaps(pp, z_lbl):
    zp_lbl = pp["dis_lbl"] * z_lbl
    Zpad = np.zeros((NROWS, D), dtype=np.float32)
    Zpad[_row_of(np.arange(NPAD)), 0] = zp_lbl
    maps = []
    for k in range(NCORES):
        c = pp["per_core"][k]
        lblk = np.arange(WPC * P) + WPC * k * P
        zpw = zp_lbl[lblk].reshape(WPC, P).T.copy()
        maps.append({
            "zpad": Zpad,
            "ilo": c["ilo"], "ihi": c["ihi"], "dloc": c["dloc"],
            "iota": IOTA, "disw": c["disw"], "zpw": zpw,
        })
    return maps


def kernel(state, edge_attr, edge_index, W1, b1, W2, b2, W3, b3):
    state = np.asarray(state)
    edge_attr = np.asarray(edge_attr)
    edge_index = np.asarray(edge_index)
    pp = _preprocess(state, edge_attr, edge_index)
    w23 = np.asarray(W2, dtype=np.float32) @ np.asarray(W3, dtype=np.float32)
    c2 = float((np.asarray(b2, dtype=np.float32) @ np.asarray(W3, dtype=np.float32)
                + np.asarray(b3, dtype=np.float32))[0])

    nc1 = build_pass1(pp["t_lo"], pp["t_hi"])
    r1 = run_bass_kernel_spmd(nc1, pass1_inmaps(pp, W1, b1, w23),
                              core_ids=list(range(NCORES)))
    z_lbl = np.zeros(NPAD, dtype=np.float32)
    for k in range(NCORES):
        z_lbl[WPC * k * P:WPC * (k + 1) * P] = r1.results[k]["z"][0][:WPC * P]

    nc2 = build_pass2(pp["t_lo"], pp["t_hi"])
    r2 = run_bass_kernel_spmd(nc2, pass2_inmaps(pp, z_lbl),
                              core_ids=list(range(NCORES)))
    y_lbl = np.zeros(NPAD, dtype=np.float32)
    for k in range(NCORES):
        y_lbl[WPC * k * P:WPC * (k + 1) * P] = r2.results[k]["y"].T.reshape(-1)
    return (y_lbl[pp["sigma"]] + c2)[:, None].astype(np.float32)


# revision 8
# speedup vs baseline: 3.9492x; 1.0621x over previous
"""Trainium2 Bass kernel for nn_CriticGCN (2-layer GCN critic, 50000 nodes,
800000 edges, 8 NeuronCores).

Algebraic reformulation (exact):
  A = S_dT diag(dis_s*dis_d) S_s + diag(dis^2)   (GCN norm adjacency)
  layer1: out1 = A @ (X W1) + b1 = ((A @ X) W1) + b1        (associativity)
  x1 = relu(out1); y = (A @ (x1 W2) + b2) @ W3 + b3
     = A @ (x1 (W2 W3)) + (b2 W3 + b3)                       (linearity)
  and A @ v = dis * (segsum_dst(dis_src * v[src]) + dis * v) per column.

So the device does two sparse passes (20-dim then 1-dim features) plus a
small dense chain; W2@W3 collapses layer 2's feature dim to 1.

Sharding: nodes are relabeled by a balance permutation and dst-sharded
across 8 cores (49 windows of 128 labels each). Edges are grouped per
(window, src<32768) and padded to fixed tile counts; per-edge rows are
fetched with dma_gather (256B rows from an HBM table), segment-summed via
one-hot matmuls on the TensorEngine accumulating in PSUM.
"""
import numpy as np
import concourse.bacc as bacc
import concourse.mybir as mybir
import concourse.tile as tile
from concourse.bass_utils import run_bass_kernel_spmd

P = 128
NCORES = 8
WPC = 49
NWIN = NCORES * WPC
NPAD = NWIN * P
GAP = 32767
NROWS = NPAD + 2
D = 64
DF = 20
GRP = 4
NGRP = (WPC + GRP - 1) // GRP
ZCOLS = NGRP * GRP * P

IOTA = np.broadcast_to(np.arange(P, dtype=np.float32)[None, :], (P, P)).copy()
IDENT = np.eye(P, dtype=np.float32)


def _row_of(lbl):
    return np.where(lbl < GAP, lbl, lbl + 1)


def _preprocess(state, edge_attr, edge_index):
    X = np.concatenate([state.reshape(-1, edge_attr.shape[1]),
                        edge_attr], 0).astype(np.float32)
    n = X.shape[0]
    src = edge_index[0].astype(np.int64)
    dst = edge_index[1].astype(np.int64)

    deg = np.bincount(dst, minlength=n) + 1
    dis = (1.0 / np.sqrt(deg)).astype(np.float32)

    order = np.argsort(-deg, kind="stable")
    sigma = np.empty(n, dtype=np.int64)
    sigma[order] = (np.arange(n) % NWIN) * P + np.arange(n) // NWIN

    s_row = _row_of(sigma[src])
    d_lbl = sigma[dst]
    d_win = d_lbl // P
    d_loc = d_lbl % P
    is_hi = s_row >= GAP + 1

    key = d_win * 2 + is_hi
    eorder = np.argsort(key, kind="stable")
    s_row_s = s_row[eorder]
    d_loc_s = d_loc[eorder]
    counts = np.bincount(key[eorder], minlength=NWIN * 2)
    off = np.concatenate([[0], np.cumsum(counts)])
    t_lo = int(np.ceil(counts[0::2].max() / P))
    t_hi = int(np.ceil(counts[1::2].max() / P))
    nlo, nhi = t_lo * P, t_hi * P

    idx_lo = np.full((NWIN, nlo), GAP, dtype=np.int64)
    idx_hi = np.full((NWIN, nhi), NPAD + 1 - (GAP + 1), dtype=np.int64)
    dloc = np.zeros((NWIN, (t_lo + t_hi) * P), dtype=np.int64)
    for w in range(NWIN):
        lo0, lo1 = off[2 * w], off[2 * w + 1]
        hi0, hi1 = off[2 * w + 1], off[2 * w + 2]
        klo, khi = lo1 - lo0, hi1 - hi0
        idx_lo[w, :klo] = s_row_s[lo0:lo1]
        idx_hi[w, :khi] = s_row_s[hi0:hi1] - (GAP + 1)
        dloc[w, :klo] = d_loc_s[lo0:lo1]
        dloc[w, nlo:nlo + khi] = d_loc_s[hi0:hi1]

    Xpad = np.zeros((NROWS, D), dtype=np.float32)
    rows = _row_of(sigma)
    Xpad[rows, :DF] = X
    Xpad[rows, DF] = dis

    dis_lbl = np.zeros(NPAD, dtype=np.float32)
    dis_lbl[sigma] = dis
    U_lbl = np.zeros((NPAD, DF), dtype=np.float32)
    U_lbl[sigma] = X * dis[:, None]

    def wrap16(a):
        return np.tile(a.astype(np.int16).reshape(-1, 16).T, (8, 1))

    per_core = []
    for k in range(NCORES):
        wr = range(WPC * k, WPC * (k + 1))
        ilo = np.concatenate([wrap16(idx_lo[w]) for w in wr], axis=1)
        ihi = np.concatenate([wrap16(idx_hi[w]) for w in wr], axis=1)
        dl = np.concatenate(
            [dloc[w].reshape(t_lo + t_hi, P).T.astype(np.float32) for w in wr],
            axis=1)
        lblk = np.arange(WPC * P) + WPC * k * P
        disw = dis_lbl[lblk].reshape(WPC, P).T.copy()
        Uw = U_lbl[lblk].reshape(WPC, P, DF).transpose(1, 0, 2).copy()
        per_core.append(dict(ilo=ilo, ihi=ihi, dloc=dl, disw=disw, U=Uw))
    return dict(per_core=per_core, Xpad=Xpad, sigma=sigma, dis_lbl=dis_lbl,
                t_lo=t_lo, t_hi=t_hi)


def build_pass1(t_lo, t_hi, reps=1):
    nlo, nhi = t_lo * P, t_hi * P
    t_w = t_lo + t_hi
    nc = bacc.Bacc("TRN2", target_bir_lowering=False, debug=False,
                   num_devices=NCORES)
    f32 = mybir.dt.float32
    xpad_d = nc.dram_tensor("xpad", [NROWS, D], f32, kind="ExternalInput")
    ilo_d = nc.dram_tensor("ilo", [128, WPC * nlo // 16], mybir.dt.int16, kind="ExternalInput")
    ihi_d = nc.dram_tensor("ihi", [128, WPC * nhi // 16], mybir.dt.int16, kind="ExternalInput")
    dloc_d = nc.dram_tensor("dloc", [P, WPC * t_w], f32, kind="ExternalInput")
    iota_d = nc.dram_tensor("iota", [P, P], f32, kind="ExternalInput")
    ident_d = nc.dram_tensor("ident", [P, P], f32, kind="ExternalInput")
    disw_d = nc.dram_tensor("disw", [P, WPC], f32, kind="ExternalInput")
    u_d = nc.dram_tensor("u", [P, WPC * DF], f32, kind="ExternalInput")
    w1_d = nc.dram_tensor("w1", [DF, 500], f32, kind="ExternalInput")
    b1_d = nc.dram_tensor("b1", [125, 4], f32, kind="ExternalInput")
    w23_d = nc.dram_tensor("w23", [125, 4], f32, kind="ExternalInput")
    z_d = nc.dram_tensor("z", [1, ZCOLS], f32, kind="ExternalOutput")

    Relu = mybir.ActivationFunctionType.Relu
    with tile.TileContext(nc) as tc:
        with tc.tile_pool(name="cst", bufs=1) as cst, \
             tc.tile_pool(name="g", bufs=5) as gpl, \
             tc.tile_pool(name="oh", bufs=3) as ohp, \
             tc.tile_pool(name="wk", bufs=3) as wk, \
             tc.tile_pool(name="ps", bufs=2, space="PSUM") as ps:
            ilo_t = cst.tile([128, WPC * nlo // 16], mybir.dt.int16)
            nc.sync.dma_start(out=ilo_t[:], in_=ilo_d[:])
            ihi_t = cst.tile([128, WPC * nhi // 16], mybir.dt.int16)
            nc.sync.dma_start(out=ihi_t[:], in_=ihi_d[:])
            dloc_t = cst.tile([P, WPC * t_w], f32)
            nc.sync.dma_start(out=dloc_t[:], in_=dloc_d[:])
            iota_t = cst.tile([P, P], f32)
            nc.sync.dma_start(out=iota_t[:], in_=iota_d[:])
            ident_t = cst.tile([P, P], f32)
            nc.sync.dma_start(out=ident_t[:], in_=ident_d[:])
            disw_t = cst.tile([P, WPC], f32)
            nc.sync.dma_start(out=disw_t[:], in_=disw_d[:])
            u_t = cst.tile([P, WPC, DF], f32)
            nc.sync.dma_start(out=u_t[:], in_=u_d[:].rearrange("p (w f) -> p w f", w=WPC))
            w1_t = cst.tile([DF, 500], f32)
            nc.sync.dma_start(out=w1_t[:], in_=w1_d[:])
            b1_t = cst.tile([125, 4], f32)
            nc.sync.dma_start(out=b1_t[:], in_=b1_d[:])
            w23_t = cst.tile([125, 4], f32)
            nc.sync.dma_start(out=w23_t[:], in_=w23_d[:])
            z_sb = cst.tile([1, ZCOLS], f32)

            for _rep in range(reps):
                for grp in range(NGRP):
                    wins = list(range(grp * GRP, min((grp + 1) * GRP, WPC)))
                    gt_ps = ps.tile([DF, GRP * P], f32, space="PSUM", tag="gtps")
                    for wi, w in enumerate(wins):
                        glo = gpl.tile([P, t_lo, D], f32, tag="glo")
                        nc.gpsimd.dma_gather(
                            out_ap=glo[:], in_ap=xpad_d[:GAP + 1, :],
                            idxs_ap=ilo_t[:, w * nlo // 16:(w + 1) * nlo // 16],
                            num_idxs=nlo, num_idxs_reg=nlo, elem_size=D,
                            single_packet=False)
                        ghi = gpl.tile([P, t_hi, D], f32, tag="ghi")
                        nc.gpsimd.dma_gather(
                            out_ap=ghi[:], in_ap=xpad_d[GAP + 1:, :],
                            idxs_ap=ihi_t[:, w * nhi // 16:(w + 1) * nhi // 16],
                            num_idxs=nhi, num_idxs_reg=nhi, elem_size=D,
                            single_packet=False)
                        slo = wk.tile([P, t_lo, DF], f32, tag="slo")
                        nc.vector.tensor_tensor(
                            out=slo[:], in0=glo[:, :, :DF],
                            in1=glo[:, :, DF:DF + 1].to_broadcast([P, t_lo, DF]),
                            op=mybir.AluOpType.mult)
                        shi = wk.tile([P, t_hi, DF], f32, tag="shi")
                        nc.vector.tensor_tensor(
                            out=shi[:], in0=ghi[:, :, :DF],
                            in1=ghi[:, :, DF:DF + 1].to_broadcast([P, t_hi, DF]),
                            op=mybir.AluOpType.mult)
                        oh = ohp.tile([P, t_w, P], f32, tag="oh")
                        nc.vector.tensor_tensor(
                            out=oh[:],
                            in0=dloc_t[:, w * t_w:(w + 1) * t_w].unsqueeze(2).to_broadcast([P, t_w, P]),
                            in1=iota_t[:].unsqueeze(1).to_broadcast([P, t_w, P]),
                            op=mybir.AluOpType.is_equal)
                        seg = ps.tile([P, DF], f32, space="PSUM", tag="seg")
                        for t in range(t_w):
                            rhs = slo[:, t, :] if t < t_lo else shi[:, t - t_lo, :]
                            nc.tensor.matmul(out=seg[:], lhsT=oh[:, t, :], rhs=rhs,
                                             start=(t == 0), stop=(t == t_w - 1))
                        g_sb = wk.tile([P, DF], f32, tag="gsb")
                        nc.vector.tensor_tensor(out=g_sb[:], in0=seg[:],
                                                in1=u_t[:, w, :],
                                                op=mybir.AluOpType.add)
                        nc.vector.tensor_tensor(
                            out=g_sb[:], in0=g_sb[:],
                            in1=disw_t[:, w:w + 1].to_broadcast([P, DF]),
                            op=mybir.AluOpType.mult)
                        nc.tensor.transpose(out=gt_ps[:, wi * P:(wi + 1) * P],
                                            in_=g_sb[:], identity=ident_t[:])
                    gt_sb = wk.tile([DF, GRP * P], f32, tag="gtsb")
                    nc.vector.tensor_copy(out=gt_sb[:], in_=gt_ps[:])
                    zrow = ps.tile([1, GRP * P], f32, space="PSUM", tag="zrow")
                    for c in range(4):
                        o1 = ps.tile([125, GRP * P], f32, space="PSUM", tag="o1")
                        nc.tensor.matmul(out=o1[:], lhsT=w1_t[:, c * 125:(c + 1) * 125],
                                         rhs=gt_sb[:], start=True, stop=True)
                        x1 = wk.tile([125, GRP * P], f32, tag="x1")
                        nc.scalar.activation(x1[:], o1[:], Relu,
                                             bias=b1_t[:, c:c + 1], scale=1.0)
                        nc.tensor.matmul(out=zrow[:], lhsT=w23_t[:, c:c + 1],
                                         rhs=x1[:], start=(c == 0), stop=(c == 3))
                    nc.vector.tensor_copy(
                        out=z_sb[:, grp * GRP * P:(grp + 1) * GRP * P], in_=zrow[:])
            nc.sync.dma_start(out=z_d[:], in_=z_sb[:])
    nc.compile()
    return nc


def build_pass2(t_lo, t_hi, reps=1):
    nlo, nhi = t_lo * P, t_hi * P
    t_w = t_lo + t_hi
    nc = bacc.Bacc("TRN2", target_bir_lowering=False, debug=False,
                   num_devices=NCORES)
    f32 = mybir.dt.float32
    zpad_d = nc.dram_tensor("zpad", [NROWS, D], f32, kind="ExternalInput")
    ilo_d = nc.dram_tensor("ilo", [128, WPC * nlo // 16], mybir.dt.int16, kind="ExternalInput")
    ihi_d = nc.dram_tensor("ihi", [128, WPC * nhi // 16], mybir.dt.int16, kind="ExternalInput")
    dloc_d = nc.dram_tensor("dloc", [P, WPC * t_w], f32, kind="ExternalInput")
    iota_d = nc.dram_tensor("iota", [P, P], f32, kind="ExternalInput")
    disw_d = nc.dram_tensor("disw", [1, WPC * P], f32, kind="ExternalInput")
    zpw_d = nc.dram_tensor("zpw", [1, WPC * P], f32, kind="ExternalInput")
    y_d = nc.dram_tensor("y", [1, WPC * P], f32, kind="ExternalOutput")

    with tile.TileContext(nc) as tc:
        with tc.tile_pool(name="cst", bufs=1) as cst, \
             tc.tile_pool(name="g", bufs=5) as gpl, \
             tc.tile_pool(name="oh", bufs=3) as ohp, \
             tc.tile_pool(name="ps", bufs=4, space="PSUM") as ps:
            ilo_t = cst.tile([128, WPC * nlo // 16], mybir.dt.int16)
            nc.sync.dma_start(out=ilo_t[:], in_=ilo_d[:])
            ihi_t = cst.tile([128, WPC * nhi // 16], mybir.dt.int16)
            nc.sync.dma_start(out=ihi_t[:], in_=ihi_d[:])
            dloc_t = cst.tile([P, WPC * t_w], f32)
            nc.sync.dma_start(out=dloc_t[:], in_=dloc_d[:])
            iota_t = cst.tile([P, P], f32)
            nc.sync.dma_start(out=iota_t[:], in_=iota_d[:])
            disw_t = cst.tile([1, WPC * P], f32)
            nc.sync.dma_start(out=disw_t[:], in_=disw_d[:])
            zpw_t = cst.tile([1, WPC * P], f32)
            nc.sync.dma_start(out=zpw_t[:], in_=zpw_d[:])
            yacc = cst.tile([1, WPC * P], f32)

            for _rep in range(reps):
                for w in range(WPC):
                    glo = gpl.tile([P, t_lo, D], f32, tag="glo")
                    nc.gpsimd.dma_gather(
                        out_ap=glo[:], in_ap=zpad_d[:GAP + 1, :],
                        idxs_ap=ilo_t[:, w * nlo // 16:(w + 1) * nlo // 16],
                        num_idxs=nlo, num_idxs_reg=nlo, elem_size=D,
                        single_packet=False)
                    ghi = gpl.tile([P, t_hi, D], f32, tag="ghi")
                    nc.gpsimd.dma_gather(
                        out_ap=ghi[:], in_ap=zpad_d[GAP + 1:, :],
                        idxs_ap=ihi_t[:, w * nhi // 16:(w + 1) * nhi // 16],
                        num_idxs=nhi, num_idxs_reg=nhi, elem_size=D,
                        single_packet=False)
                    oh = ohp.tile([P, t_w, P], f32, tag="oh")
                    nc.vector.tensor_tensor(
                        out=oh[:],
                        in0=dloc_t[:, w * t_w:(w + 1) * t_w].unsqueeze(2).to_broadcast([P, t_w, P]),
                        in1=iota_t[:].unsqueeze(1).to_broadcast([P, t_w, P]),
                        op=mybir.AluOpType.is_equal)
                    # data-stationary / one-hot-moving: 1-col LDWEIGHTS and a
                    # 128-wide moving stream amortize per-matmul overheads.
                    seg = ps.tile([1, P], f32, space="PSUM", tag="seg")
                    for t in range(t_w):
                        lhs = glo[:, t, 0:1] if t < t_lo else ghi[:, t - t_lo, 0:1]
                        nc.tensor.matmul(out=seg[:], lhsT=lhs, rhs=oh[:, t, :],
                                         start=(t == 0), stop=(t == t_w - 1))
                    nc.vector.tensor_copy(out=yacc[:, w * P:(w + 1) * P], in_=seg[:])
            yout = cst.tile([1, WPC * P], f32)
            nc.vector.tensor_tensor(out=yout[:], in0=yacc[:], in1=zpw_t[:],
                                    op=mybir.AluOpType.add)
            nc.vector.tensor_tensor(out=yout[:], in0=yout[:], in1=disw_t[:],
                                    op=mybir.AluOpType.mult)
            nc.sync.dma_start(out=y_d[:], in_=yout[:])
    nc.compile()
    return nc


def pass1_inmaps(pp, W1, b1, w23):
    maps = []
    for k in range(NCORES):
        c = pp["per_core"][k]
        maps.append({
            "xpad": pp["Xpad"],
            "ilo": c["ilo"], "ihi": c["ihi"], "dloc": c["dloc"],
            "iota": IOTA, "ident": IDENT,
            "disw": c["disw"],
            "u": c["U"].reshape(P, WPC * DF),
            "w1": np.ascontiguousarray(W1, dtype=np.float32),
            "b1": np.asarray(b1, dtype=np.float32).reshape(4, 125).T.copy(),
            "w23": np.asarray(w23, dtype=np.float32).reshape(4, 125).T.copy(),
        })
    return maps


def pass2_inmaps(pp, z_lbl):
    zp_lbl = pp["dis_lbl"] * z_lbl
    Zpad = np.zeros((NROWS, D), dtype=np.float32)
    Zpad[_row_of(np.arange(NPAD)), 0] = zp_lbl
    maps = []
    for k in range(NCORES):
        c = pp["per_core"][k]
        lblk = np.arange(WPC * P) + WPC * k * P
        maps.append({
            "zpad": Zpad,
            "ilo": c["ilo"], "ihi": c["ihi"], "dloc": c["dloc"],
            "iota": IOTA,
            "disw": pp["dis_lbl"][lblk][None, :].copy(),
            "zpw": zp_lbl[lblk][None, :].copy(),
        })
    return maps


def kernel(state, edge_attr, edge_index, W1, b1, W2, b2, W3, b3):
    state = np.asarray(state)
    edge_attr = np.asarray(edge_attr)
    edge_index = np.asarray(edge_index)
    pp = _preprocess(state, edge_attr, edge_index)
    w23 = np.asarray(W2, dtype=np.float32) @ np.asarray(W3, dtype=np.float32)
    c2 = float((np.asarray(b2, dtype=np.float32) @ np.asarray(W3, dtype=np.float32)
                + np.asarray(b3, dtype=np.float32))[0])

    nc1 = build_pass1(pp["t_lo"], pp["t_hi"])
    r1 = run_bass_kernel_spmd(nc1, pass1_inmaps(pp, W1, b1, w23),
                              core_ids=list(range(NCORES)))
    z_lbl = np.zeros(NPAD, dtype=np.float32)
    for k in range(NCORES):
        z_lbl[WPC * k * P:WPC * (k + 1) * P] = r1.results[k]["z"][0][:WPC * P]

    nc2 = build_pass2(pp["t_lo"], pp["t_hi"])
    r2 = run_bass_kernel_spmd(nc2, pass2_inmaps(pp, z_lbl),
                              core_ids=list(range(NCORES)))
    y_lbl = np.zeros(NPAD, dtype=np.float32)
    for k in range(NCORES):
        y_lbl[WPC * k * P:WPC * (k + 1) * P] = r2.results[k]["y"][0]
    return (y_lbl[pp["sigma"]] + c2)[:, None].astype(np.float32)
